# revision 7
# baseline (speedup 1.0000x reference)
"""TRN2 Bass kernel for CGCNN-style gated graph conv (nn_ConvLayer_36395552866974).

Strategy (8-core SPMD, graph parallelism):
  - Host: sort edges by destination node (indices1), group destination nodes
    into 128-node blocks, pad each block's edge segment to TPB*128 slots so
    every 128-edge tile belongs to exactly one destination block.
    Blocks are sharded contiguously across the 8 cores (49 blocks/core) so the
    scatter-add is core-local: NO collective needed.
  - Host prepares, per core, the transposed edge-feature stream
    vT = [sites[d1] | sites[d2] | bonds | 1]^T in bf16 (three K-chunks of
    128/128/65 rows) plus a per-tile one-hot scatter matrix (bf16) and the f32
    residual site rows.
  - Device per 128-edge tile: 3 matmuls (K=128,128,65; N=256) accumulate
    pre-activations for sigmoid||relu paths into PSUM; ACT computes sigmoid,
    DVE computes relu and the product; one one-hot matmul scatter-adds the
    gated messages into the block's PSUM aggregate; per block, DVE adds the
    f32 residual and the result is DMA'd out.
"""

import sys

sys.path.insert(0, "/opt/trn_rl_repo")

import numpy as np
import ml_dtypes

import concourse.bacc as bacc
import concourse.mybir as mybir
import concourse.tile as tile
from concourse.bass_utils import run_bass_kernel_spmd

BF16 = ml_dtypes.bfloat16

P = 128           # tile size in edges / node-block size
NCORES = 8
S = 128           # site feature dim
BD = 64           # bond feature dim
KC = [128, 128, BD + 1]  # contraction chunks (s1, s2, bonds+bias-ones)


def _build(nb_core, tpb, tiles_per_slab, QUAD):
    """Build the SPMD Bass program. nb_core: node blocks per core; tpb: tiles
    per block; tiles_per_slab must divide nb_core*tpb."""
    T = nb_core * tpb            # tiles per core
    SLOTS = T * P                # edge slots per core
    assert T % tiles_per_slab == 0 and tiles_per_slab % QUAD == 0

    nc = bacc.Bacc("TRN2", target_bir_lowering=False, debug=False)
    dt = mybir.dt
    v1 = nc.dram_tensor("v1", [128, SLOTS], dt.bfloat16, kind="ExternalInput")
    v2 = nc.dram_tensor("v2", [128, SLOTS], dt.bfloat16, kind="ExternalInput")
    v3 = nc.dram_tensor("v3", [KC[2], SLOTS], dt.bfloat16, kind="ExternalInput")
    oh = nc.dram_tensor("oh", [128, SLOTS], dt.float8e4, kind="ExternalInput")
    w1 = nc.dram_tensor("w1", [128, 2 * S], dt.bfloat16, kind="ExternalInput")
    w2 = nc.dram_tensor("w2", [128, 2 * S], dt.bfloat16, kind="ExternalInput")
    w3 = nc.dram_tensor("w3", [KC[2], 2 * S], dt.bfloat16, kind="ExternalInput")
    res = nc.dram_tensor("res", [nb_core * P, S], dt.float32, kind="ExternalInput")
    out = nc.dram_tensor("out", [nb_core * P, S], dt.float32, kind="ExternalOutput")

    with tile.TileContext(nc) as tc:
        with (
            tc.tile_pool(name="wsb", bufs=1) as wsb,
            tc.tile_pool(name="slab", bufs=2) as slab,
            tc.tile_pool(name="act", bufs=3) as actp,
            tc.tile_pool(name="resp", bufs=2) as resp,
            tc.tile_pool(name="qps", bufs=2, space="PSUM") as qps,
            tc.tile_pool(name="aps", bufs=2, space="PSUM") as aps,
        ):
            w1_t = wsb.tile([128, 2 * S], dt.bfloat16, tag="w1")
            w2_t = wsb.tile([128, 2 * S], dt.bfloat16, tag="w2")
            w3_t = wsb.tile([KC[2], 2 * S], dt.bfloat16, tag="w3")
            nc.sync.dma_start(w1_t[:], w1[:])
            nc.sync.dma_start(w2_t[:], w2[:])
            nc.sync.dma_start(w3_t[:], w3[:])

            SLAB_E = tiles_per_slab * P
            v1_s = v2_s = v3_s = oh_s = None
            quad = None
            sig = rel = gat = None
            agg = None
            res_t = None

            for t in range(T):
                ts = t % tiles_per_slab
                if ts == 0:
                    s0 = (t // tiles_per_slab) * SLAB_E
                    v1_s = slab.tile([128, SLAB_E], dt.bfloat16, tag="v1s")
                    v2_s = slab.tile([128, SLAB_E], dt.bfloat16, tag="v2s")
                    v3_s = slab.tile([KC[2], SLAB_E], dt.bfloat16, tag="v3s")
                    oh_s = slab.tile([128, SLAB_E], dt.float8e4, tag="ohs")
                    nc.sync.dma_start(v1_s[:], v1[:, s0 : s0 + SLAB_E])
                    nc.sync.dma_start(v2_s[:], v2[:, s0 : s0 + SLAB_E])
                    nc.sync.dma_start(v3_s[:], v3[:, s0 : s0 + SLAB_E])
                    nc.sync.dma_start(oh_s[:], oh[:, s0 : s0 + SLAB_E])

                q = t % QUAD
                if q == 0:
                    quad = qps.tile([P, QUAD * 2 * S], dt.float32, space="PSUM", tag="quad")

                c0 = q * 2 * S
                esl = slice(ts * P, (ts + 1) * P)
                nc.tensor.matmul(quad[:, c0 : c0 + 2 * S], lhsT=v1_s[:, esl],
                                 rhs=w1_t[:], start=True, stop=False)
                nc.tensor.matmul(quad[:, c0 : c0 + 2 * S], lhsT=v2_s[:, esl],
                                 rhs=w2_t[:], start=False, stop=False)
                nc.tensor.matmul(quad[:, c0 : c0 + 2 * S], lhsT=v3_s[:, esl],
                                 rhs=w3_t[:], start=False, stop=True)

                if q == QUAD - 1:
                    # quad viewed as [P, QUAD, 2S]: sigmoid on [:, :, :S], relu on [:, :, S:]
                    q3 = quad[:].rearrange("p (a b) -> p a b", b=2 * S)
                    sig = actp.tile([P, QUAD * S], dt.float32, tag="sig")
                    rel = actp.tile([P, QUAD * S], dt.float32, tag="rel")
                    gat = actp.tile([P, QUAD * S], dt.bfloat16, tag="gat")
                    sig3 = sig[:].rearrange("p (a b) -> p a b", b=S)
                    rel3 = rel[:].rearrange("p (a b) -> p a b", b=S)
                    nc.scalar.activation(sig3, q3[:, :, 0:S],
                                         mybir.ActivationFunctionType.Sigmoid)
                    nc.vector.tensor_scalar_max(rel3, q3[:, :, S : 2 * S], 0.0)
                    nc.vector.tensor_tensor(gat[:], sig[:], rel[:],
                                            op=mybir.AluOpType.mult)
                    # scatter the QUAD completed tiles
                    for tt in range(t - QUAD + 1, t + 1):
                        blk = tt // tpb
                        i_in_b = tt % tpb
                        if i_in_b == 0:
                            agg = aps.tile([P, S], dt.float32, space="PSUM", tag="agg")
                        tts = slice((tt % tiles_per_slab) * P, (tt % tiles_per_slab + 1) * P)
                        gsl = slice((tt % QUAD) * S, (tt % QUAD + 1) * S)
                        nc.tensor.matmul(agg[:], lhsT=oh_s[:, tts], rhs=gat[:, gsl],
                                         start=(i_in_b == 0), stop=(i_in_b == tpb - 1))
                        if i_in_b == tpb - 1:
                            res_t = resp.tile([P, S], dt.float32, tag="res")
                            nc.sync.dma_start(res_t[:], res[blk * P : (blk + 1) * P, :])
                            o_t = resp.tile([P, S], dt.float32, tag="out")
                            nc.vector.tensor_add(o_t[:], agg[:], res_t[:])
                            nc.sync.dma_start(out[blk * P : (blk + 1) * P, :], o_t[:])
    nc.compile()
    return nc


# ---------------------------------------------------------------- host side

# Full-problem constants (hardcoded per harness contract)
N_FULL, E_FULL = 50000, 800000


def _prep(sites, bonds, W_sig, b_sig, W_soft, b_soft, indices1, indices2,
          nb_core, tpb, tiles_per_slab, nblk, ncores):
    """Host-side shard/layout prep. Returns (in_maps, order, nb_core, node_cap)."""
    N = sites.shape[0]
    E = bonds.shape[0]
    d1 = np.asarray(indices1).astype(np.int64)
    d2 = np.asarray(indices2).astype(np.int64)
    order = np.argsort(d1, kind="stable")
    d1s, d2s = d1[order], d2[order]

    T = nb_core * tpb
    SLOTS = T * P
    cnt = np.bincount(d1s // P, minlength=nblk)
    assert cnt.max() <= tpb * P, f"block overflow: {cnt.max()} > {tpb * P}"
    starts = np.zeros(nblk, np.int64)
    starts[1:] = np.cumsum(cnt)[:-1]
    within = np.arange(E) - starts[d1s // P]
    slot = (d1s // P) * (tpb * P) + within  # global slot id

    sites_b = sites.astype(BF16)
    bonds_b = bonds.astype(BF16)

    # global slot-indexed arrays
    S_all = nblk * tpb * P
    v1g = np.zeros((S_all, S), BF16)
    v2g = np.zeros((S_all, S), BF16)
    v3g = np.zeros((S_all, KC[2]), BF16)
    ohg = np.zeros((S_all, P), ml_dtypes.float8_e4m3)
    v1g[slot] = sites_b[d1s]
    v2g[slot] = sites_b[d2s]
    v3g[slot, :BD] = bonds_b[order]
    v3g[:, BD] = BF16(1.0)
    ohg[slot, d1s % P] = ml_dtypes.float8_e4m3(1.0)

    w1 = np.concatenate([W_sig[0:128], W_soft[0:128]], axis=1).astype(BF16)
    w2 = np.concatenate([W_sig[128:256], W_soft[128:256]], axis=1).astype(BF16)
    w3 = np.zeros((KC[2], 2 * S), np.float32)
    w3[:BD, :S] = W_sig[256:]
    w3[:BD, S:] = W_soft[256:]
    w3[BD, :S] = b_sig
    w3[BD, S:] = b_soft
    w3 = w3.astype(BF16)

    node_cap = nblk * P
    res_g = np.zeros((node_cap, S), np.float32)
    res_g[:N] = sites.astype(np.float32)

    in_maps = []
    for c in range(ncores):
        b0 = c * nb_core
        sl = slice(b0 * tpb * P, (b0 + nb_core) * tpb * P)
        nsl = slice(b0 * P, (b0 + nb_core) * P)
        T_core = nb_core * tpb
        oh_c = ohg[sl].reshape(T_core, P, P).transpose(1, 0, 2).reshape(P, T_core * P)
        in_maps.append({
            "v1": np.ascontiguousarray(v1g[sl].T),
            "v2": np.ascontiguousarray(v2g[sl].T),
            "v3": np.ascontiguousarray(v3g[sl].T),
            "oh": np.ascontiguousarray(oh_c),
            "w1": w1, "w2": w2, "w3": w3,
            "res": res_g[nsl],
        })
    return in_maps


def kernel(sites, bonds, W_sig, b_sig, W_soft, b_soft, indices1, indices2,
           _debug_cfg=None, _trace=False):
    """Full inputs in, full output out. Shards internally across 8 NeuronCores."""
    sites = np.asarray(sites)
    bonds = np.asarray(bonds)
    B = sites.shape[0]
    s2 = sites.reshape(-1, sites.shape[-1])
    b2 = bonds.reshape(-1, bonds.shape[-1])
    N, E = s2.shape[0], b2.shape[0]

    ncores = NCORES
    nblk = -(-N // P)  # ceil
    nb_core = -(-nblk // ncores)
    nblk = nb_core * ncores  # pad block count
    cnt = np.bincount(np.asarray(indices1).astype(np.int64) // P, minlength=nblk)
    tpb = max(2, int(-(-cnt.max() // P)))
    if tpb % 2:
        tpb += 1  # keep tiles_per_slab divisibility simple
    T = nb_core * tpb
    QUAD = 6 if T % 6 == 0 else 2
    tiles_per_slab = QUAD
    for cand in range(48, QUAD - 1, -1):
        if cand % QUAD == 0 and T % cand == 0:
            tiles_per_slab = cand
            break

    if _debug_cfg is not None:
        nb_core, tpb, tiles_per_slab, QUAD = _debug_cfg  # small-scale testing
        T = nb_core * tpb
    assert T % tiles_per_slab == 0, (T, tiles_per_slab)

    in_maps = _prep(s2, b2, np.asarray(W_sig), np.asarray(b_sig),
                    np.asarray(W_soft), np.asarray(b_soft),
                    indices1, indices2, nb_core, tpb, tiles_per_slab,
                    nblk, ncores)
    nc = _build(nb_core, tpb, tiles_per_slab, QUAD)
    kw = {}
    if _trace:
        kw = dict(trace=True)
    import time as _time
    _t0 = _time.perf_counter()
    r = run_bass_kernel_spmd(nc, in_maps, core_ids=list(range(ncores)), **kw)
    kernel._last_run_s = _time.perf_counter() - _t0
    outs = [r.results[c]["out"] for c in range(ncores)]
    full = np.concatenate(outs, axis=0)[:N]
    out = full.reshape(B, N, -1).astype(np.float32)
    kernel._last_exec_ns = r.exec_time_ns
    return out


# revision 8
# speedup vs baseline: 1.1023x; 1.1023x over previous
"""TRN2 Bass kernel for CGCNN-style gated graph conv (nn_ConvLayer_36395552866974).

Strategy (8-core SPMD, graph parallelism):
  - Host: sort edges by destination node (indices1), group destination nodes
    into 128-node blocks, pad each block's edge segment to TPB*128 slots so
    every 128-edge tile belongs to exactly one destination block.
    Blocks are sharded contiguously across the 8 cores (49 blocks/core) so the
    scatter-add is core-local: NO collective needed.
  - Host prepares, per core, the transposed edge-feature stream
    vT = [sites[d1] | sites[d2] | bonds | 1]^T in bf16 (three K-chunks of
    128/128/65 rows) plus a per-tile one-hot scatter matrix (bf16) and the f32
    residual site rows.
  - Device per 128-edge tile: 3 matmuls (K=128,128,65; N=256) accumulate
    pre-activations for sigmoid||relu paths into PSUM; ACT computes sigmoid,
    DVE computes relu and the product; one one-hot matmul scatter-adds the
    gated messages into the block's PSUM aggregate; per block, DVE adds the
    f32 residual and the result is DMA'd out.
"""

import sys

sys.path.insert(0, "/opt/trn_rl_repo")

import numpy as np
import ml_dtypes

import concourse.bacc as bacc
import concourse.mybir as mybir
import concourse.tile as tile
from concourse.bass_utils import run_bass_kernel_spmd

BF16 = ml_dtypes.bfloat16

P = 128           # tile size in edges / node-block size
NCORES = 8
S = 128           # site feature dim
BD = 64           # bond feature dim
KC = [128, 128, BD + 1]  # contraction chunks (s1, s2, bonds+bias-ones)


def _build(nb_core, tpb, tiles_per_slab, QUAD):
    """Build the SPMD Bass program. nb_core: node blocks per core; tpb: tiles
    per block; tiles_per_slab must divide nb_core*tpb."""
    T = nb_core * tpb            # tiles per core
    SLOTS = T * P                # edge slots per core
    assert T % tiles_per_slab == 0 and tiles_per_slab % QUAD == 0

    nc = bacc.Bacc("TRN2", target_bir_lowering=False, debug=False)
    dt = mybir.dt
    v1 = nc.dram_tensor("v1", [128, SLOTS], dt.bfloat16, kind="ExternalInput")
    v2 = nc.dram_tensor("v2", [128, SLOTS], dt.bfloat16, kind="ExternalInput")
    v3 = nc.dram_tensor("v3", [KC[2], SLOTS], dt.bfloat16, kind="ExternalInput")
    oh = nc.dram_tensor("oh", [128, SLOTS], dt.float8e4, kind="ExternalInput")
    w1 = nc.dram_tensor("w1", [128, 2 * S], dt.bfloat16, kind="ExternalInput")
    w2 = nc.dram_tensor("w2", [128, 2 * S], dt.bfloat16, kind="ExternalInput")
    w3 = nc.dram_tensor("w3", [KC[2], 2 * S], dt.bfloat16, kind="ExternalInput")
    res = nc.dram_tensor("res", [nb_core * P, S], dt.float32, kind="ExternalInput")
    out = nc.dram_tensor("out", [nb_core * P, S], dt.float32, kind="ExternalOutput")

    with tile.TileContext(nc) as tc:
        with (
            tc.tile_pool(name="wsb", bufs=1) as wsb,
            tc.tile_pool(name="slab", bufs=2) as slab,
            tc.tile_pool(name="act", bufs=3) as actp,
            tc.tile_pool(name="resp", bufs=2) as resp,
            tc.tile_pool(name="qps", bufs=2, space="PSUM") as qps,
            tc.tile_pool(name="aps", bufs=2, space="PSUM") as aps,
        ):
            w1_t = wsb.tile([128, 2 * S], dt.bfloat16, tag="w1")
            w2_t = wsb.tile([128, 2 * S], dt.bfloat16, tag="w2")
            w3_t = wsb.tile([KC[2], 2 * S], dt.bfloat16, tag="w3")
            nc.sync.dma_start(w1_t[:], w1[:])
            nc.sync.dma_start(w2_t[:], w2[:])
            nc.sync.dma_start(w3_t[:], w3[:])

            SLAB_E = tiles_per_slab * P
            v1_s = v2_s = v3_s = oh_s = None
            quad = None
            sig = rel = gat = None
            agg = None
            res_t = None

            for t in range(T):
                ts = t % tiles_per_slab
                if ts == 0:
                    s0 = (t // tiles_per_slab) * SLAB_E
                    v1_s = slab.tile([128, SLAB_E], dt.bfloat16, tag="v1s")
                    v2_s = slab.tile([128, SLAB_E], dt.bfloat16, tag="v2s")
                    v3_s = slab.tile([KC[2], SLAB_E], dt.bfloat16, tag="v3s")
                    oh_s = slab.tile([128, SLAB_E], dt.float8e4, tag="ohs")
                    nc.sync.dma_start(v1_s[:], v1[:, s0 : s0 + SLAB_E])
                    nc.sync.dma_start(v2_s[:], v2[:, s0 : s0 + SLAB_E])
                    nc.sync.dma_start(v3_s[:], v3[:, s0 : s0 + SLAB_E])
                    nc.sync.dma_start(oh_s[:], oh[:, s0 : s0 + SLAB_E])

                q = t % QUAD
                if q == 0:
                    quad = qps.tile([P, QUAD * 2 * S], dt.float32, space="PSUM", tag="quad")

                c0 = q * 2 * S
                esl = slice(ts * P, (ts + 1) * P)
                nc.tensor.matmul(quad[:, c0 : c0 + 2 * S], lhsT=v1_s[:, esl],
                                 rhs=w1_t[:], start=True, stop=False)
                nc.tensor.matmul(quad[:, c0 : c0 + 2 * S], lhsT=v2_s[:, esl],
                                 rhs=w2_t[:], start=False, stop=False)
                nc.tensor.matmul(quad[:, c0 : c0 + 2 * S], lhsT=v3_s[:, esl],
                                 rhs=w3_t[:], start=False, stop=True)

                if q == QUAD - 1:
                    # quad viewed as [P, QUAD, 2S]: sigmoid on [:, :, :S], relu on [:, :, S:]
                    q3 = quad[:].rearrange("p (a b) -> p a b", b=2 * S)
                    sig = actp.tile([P, QUAD * S], dt.float32, tag="sig")
                    rel = actp.tile([P, QUAD * S], dt.float32, tag="rel")
                    gat = actp.tile([P, QUAD * S], dt.bfloat16, tag="gat")
                    sig3 = sig[:].rearrange("p (a b) -> p a b", b=S)
                    rel3 = rel[:].rearrange("p (a b) -> p a b", b=S)
                    nc.scalar.activation(sig3, q3[:, :, 0:S],
                                         mybir.ActivationFunctionType.Sigmoid)
                    nc.vector.tensor_scalar_max(rel3, q3[:, :, S : 2 * S], 0.0)
                    nc.vector.tensor_tensor(gat[:], sig[:], rel[:],
                                            op=mybir.AluOpType.mult)
                    # scatter the QUAD completed tiles
                    for tt in range(t - QUAD + 1, t + 1):
                        blk = tt // tpb
                        i_in_b = tt % tpb
                        if i_in_b == 0:
                            agg = aps.tile([P, S], dt.float32, space="PSUM", tag="agg")
                        tts = slice((tt % tiles_per_slab) * P, (tt % tiles_per_slab + 1) * P)
                        gsl = slice((tt % QUAD) * S, (tt % QUAD + 1) * S)
                        nc.tensor.matmul(agg[:], lhsT=oh_s[:, tts], rhs=gat[:, gsl],
                                         start=(i_in_b == 0), stop=(i_in_b == tpb - 1))
                        if i_in_b == tpb - 1:
                            res_t = resp.tile([P, S], dt.float32, tag="res")
                            nc.sync.dma_start(res_t[:], res[blk * P : (blk + 1) * P, :])
                            o_t = resp.tile([P, S], dt.float32, tag="out")
                            nc.vector.tensor_add(o_t[:], agg[:], res_t[:])
                            nc.sync.dma_start(out[blk * P : (blk + 1) * P, :], o_t[:])
    nc.compile()
    return nc


# ---------------------------------------------------------------- host side

# Full-problem constants (hardcoded per harness contract)
N_FULL, E_FULL = 50000, 800000


def _prep(sites, bonds, W_sig, b_sig, W_soft, b_soft, indices1, indices2,
          nb_core, tpb, tiles_per_slab, nblk, ncores, L):
    """Host-side shard/layout prep. L: node id -> balanced local id."""
    N = sites.shape[0]
    E = bonds.shape[0]
    d1 = np.asarray(indices1).astype(np.int64)
    d2 = np.asarray(indices2).astype(np.int64)
    d1L = L[d1]
    order = np.argsort(d1L, kind="stable")
    d1s, d2s = d1[order], d2[order]
    d1Ls = d1L[order]

    T = nb_core * tpb
    SLOTS = T * P
    cnt = np.bincount(d1Ls // P, minlength=nblk)
    assert cnt.max() <= tpb * P, f"block overflow: {cnt.max()} > {tpb * P}"
    starts = np.zeros(nblk, np.int64)
    starts[1:] = np.cumsum(cnt)[:-1]
    within = np.arange(E) - starts[d1Ls // P]
    slot = (d1Ls // P) * (tpb * P) + within  # global slot id

    sites_b = sites.astype(BF16)
    bonds_b = bonds.astype(BF16)

    # global slot-indexed arrays
    S_all = nblk * tpb * P
    v1g = np.zeros((S_all, S), BF16)
    v2g = np.zeros((S_all, S), BF16)
    v3g = np.zeros((S_all, KC[2]), BF16)
    ohg = np.zeros((S_all, P), ml_dtypes.float8_e4m3)
    v1g[slot] = sites_b[d1s]
    v2g[slot] = sites_b[d2s]
    v3g[slot, :BD] = bonds_b[order]
    v3g[:, BD] = BF16(1.0)
    ohg[slot, d1Ls % P] = ml_dtypes.float8_e4m3(1.0)

    w1 = np.concatenate([W_sig[0:128], W_soft[0:128]], axis=1).astype(BF16)
    w2 = np.concatenate([W_sig[128:256], W_soft[128:256]], axis=1).astype(BF16)
    w3 = np.zeros((KC[2], 2 * S), np.float32)
    w3[:BD, :S] = W_sig[256:]
    w3[:BD, S:] = W_soft[256:]
    w3[BD, :S] = b_sig
    w3[BD, S:] = b_soft
    w3 = w3.astype(BF16)

    node_cap = nblk * P
    res_g = np.zeros((node_cap, S), np.float32)
    res_g[L[:N]] = sites.astype(np.float32)

    in_maps = []
    for c in range(ncores):
        b0 = c * nb_core
        sl = slice(b0 * tpb * P, (b0 + nb_core) * tpb * P)
        nsl = slice(b0 * P, (b0 + nb_core) * P)
        T_core = nb_core * tpb
        oh_c = ohg[sl].reshape(T_core, P, P).transpose(1, 0, 2).reshape(P, T_core * P)
        in_maps.append({
            "v1": np.ascontiguousarray(v1g[sl].T),
            "v2": np.ascontiguousarray(v2g[sl].T),
            "v3": np.ascontiguousarray(v3g[sl].T),
            "oh": np.ascontiguousarray(oh_c),
            "w1": w1, "w2": w2, "w3": w3,
            "res": res_g[nsl],
        })
    return in_maps


def kernel(sites, bonds, W_sig, b_sig, W_soft, b_soft, indices1, indices2,
           _debug_cfg=None, _trace=False):
    """Full inputs in, full output out. Shards internally across 8 NeuronCores."""
    sites = np.asarray(sites)
    bonds = np.asarray(bonds)
    B = sites.shape[0]
    s2 = sites.reshape(-1, sites.shape[-1])
    b2 = bonds.reshape(-1, bonds.shape[-1])
    N, E = s2.shape[0], b2.shape[0]

    ncores = NCORES
    nblk = -(-N // P)  # ceil
    nb_core = -(-nblk // ncores)
    nblk = nb_core * ncores  # pad block count

    # degree-balanced node -> (block, slot) assignment: minimizes the max
    # per-block edge load, hence the padded tiles-per-block
    import heapq
    d1a = np.asarray(indices1).astype(np.int64).reshape(-1)
    deg = np.bincount(d1a, minlength=nblk * P)
    norder = np.argsort(-deg, kind="stable")
    loads = np.zeros(nblk, np.int64)
    nslots = np.zeros(nblk, np.int64)
    assign = np.empty(nblk * P, np.int64)
    npos = np.empty(nblk * P, np.int64)
    h = [(0, b) for b in range(nblk)]
    heapq.heapify(h)
    for n in norder:
        while True:
            l, b = heapq.heappop(h)
            if nslots[b] < P:
                break
        assign[n] = b
        npos[n] = nslots[b]
        nslots[b] += 1
        loads[b] = l + deg[n]
        if nslots[b] < P:
            heapq.heappush(h, (loads[b], b))
    L = assign * P + npos

    QUAD = 4
    tpb = max(QUAD, int(-(-loads.max() // P)))
    tpb += (-tpb) % QUAD  # round up to multiple of QUAD
    T = nb_core * tpb
    tiles_per_slab = QUAD
    for cand in range(48, QUAD - 1, -1):
        if cand % QUAD == 0 and T % cand == 0:
            tiles_per_slab = cand
            break

    if _debug_cfg is not None:
        nb_core, tpb, tiles_per_slab, QUAD = _debug_cfg  # small-scale testing
        T = nb_core * tpb
    assert T % tiles_per_slab == 0 and T % QUAD == 0, (T, tiles_per_slab, QUAD)

    in_maps = _prep(s2, b2, np.asarray(W_sig), np.asarray(b_sig),
                    np.asarray(W_soft), np.asarray(b_soft),
                    indices1, indices2, nb_core, tpb, tiles_per_slab,
                    nblk, ncores, L)
    nc = _build(nb_core, tpb, tiles_per_slab, QUAD)
    kw = {}
    if _trace:
        kw = dict(trace=True)
    import time as _time
    _t0 = _time.perf_counter()
    r = run_bass_kernel_spmd(nc, in_maps, core_ids=list(range(ncores)), **kw)
    kernel._last_run_s = _time.perf_counter() - _t0
    outs = [r.results[c]["out"] for c in range(ncores)]
    full = np.concatenate(outs, axis=0)
    out = full[L[:N]].reshape(B, N, -1).astype(np.float32)
    kernel._last_exec_ns = r.exec_time_ns
    return out
